# revision 34
# baseline (speedup 1.0000x reference)
"""Trainium2 Bass kernel for nn_CombineNode_7395933684091 (gnn_message_passing).

Hierarchy: 128 leaf terms (each D=1024 -> H=32), 16 internal terms
(concat of 8 children hiddens, 256 -> 32), 1 root (concat of 16
internal hiddens, 512 -> 32); every term also has a 1-dim predict head.
All matmuls followed by tanh.

Strategy: data-parallel over batch across 8 cores (Bc = 1024 rows per
core), weights replicated. On-chip layout keeps hidden features on the
PARTITION axis ("h^T layout": tiles are [features, batch]), so every
level's contraction is a natural PE matmul and the child-concat is just
stacking partition tiles. x and all weights are repacked on the host so
every DMA is contiguous per partition.

Leaf level: 4 panels x 8 groups (4 leaves) x 8 k-chunk accumulated
[128,128]x[128,512] matmuls. The per-term predict heads ride along as
extra block-diagonal columns fused into the internal-level stationary
operand (cw) and the root-level stationary operand (rw2), so they cost
no extra PE streaming.

Matmul operands are float16: same PE stream rate as f32r (1 col/cycle)
but enables Fast Weight Load (fp32 disables FWL) so LDWEIGHTS hides
behind the matmul stream, and halves HBM + SBUF traffic. fp16's 10
mantissa bits keep the end-to-end max abs error ~1.6e-3 (vs 2e-2 gate).

Scheduling notes (perfetto-driven, 153.0us -> 151.2us):
- PE pre-warm uses the bf16 const AP (an fp32 const forces LOW_HIGH
  2-pass matmuls) and is sized to end when the first x/weight chunks
  land (~10.3us).
- Each dma_start costs ~600ns of issue time; early per-128KB transfers
  sustain only ~140GB/s per queue (~280 aggregate, global ~8-deep
  completion-sem ring), and the 16KB-run panel loads (400+GB/s) crush
  anything concurrent — the preamble order keeps every latency-critical
  load ahead of wp1-3 and is at the measured supply frontier. cc's tiny
  220B packets go after x-k0, not in front.
- Combine matmuls are deferred by half a leaf group (a FIFO popped
  twice per group) so the ~460ns leaf-tanh latency never stalls PE;
  this made the 43-159us stream region completely gapless.
- Scalar ACTIVATE has ~300ns fixed cost: root hidden + int predicts are
  merged into ONE 48-row tanh (rhi), and the endgame runs node 15 as
  384+128 col segments with each segment's root chain emitted under
  later leaf blocks; final flushes split across the sync+scalar queues.
"""

import numpy as np

B, D, H = 8192, 1024, 32
L, I, CPI = 128, 16, 8
NCORES = 8
BC = B // NCORES      # 1024 batch rows per core
BN = 512              # batch tile width (one PSUM bank of f32)
NBH = BC // BN        # 2 batch halves
KC = D // 128         # 8 contraction chunks for the leaf level
NPANEL = 4            # leaf panels (8 groups of 4 leaves each)
GPP = 8               # groups per panel
NOUT = L + I + 1      # 145
NWARM = 5             # pre-warm matmuls (512 cols each)

MM_DT = "float16"

_CACHE = {}


def _build_nc():
    from contextlib import ExitStack

    import concourse.mybir as mybir
    import concourse.tile as tile
    from concourse import bacc

    f32 = mybir.dt.float32
    bf16 = mybir.dt.bfloat16
    Tanh = mybir.ActivationFunctionType.Tanh
    mmdt = getattr(mybir.dt, MM_DT)

    nc = bacc.Bacc("TRN2", target_bir_lowering=False, debug=False)

    # x, host-packed partition-major so every early DMA has 2-4KB
    # contiguous runs per partition (1KB-run transfers cap at
    # ~140GB/s/queue; long-run chains reach 250-400GB/s):
    # cols 0:4096 = bn0 half, k-major ([p][k][c<512]); 4096:8192 = bn1
    xtp = nc.dram_tensor("xtp", [128, KC * 1024], mmdt, kind="ExternalInput")
    # leaf weights, wave-major per panel: [p][pp][half*4096 + k*512 +
    # gl%4*128 + c] — same layout as the wp SBUF tiles so the panel-0
    # per-pair DMAs are contiguous on BOTH sides (big packets)
    lwh = nc.dram_tensor("lwh", [NPANEL, 128, KC * 1024], mmdt,
                         kind="ExternalInput")
    # fused internal-trans + leaf-predict stationary: per (node i, chunk j)
    # a [128, 128] block: cols 0:32 int_W chunk, col 32+4j+c leaf Wp diag,
    # rest zero padding (full-width stationaries keep LDW pull-ahead alive)
    cw = nc.dram_tensor("cw", [128, I * 2 * 128], mmdt, kind="ExternalInput")
    # fused root-trans + int-predict stationary: per panel q a [128, 128]
    # block (cols 0:32 root_W chunk, 32:48 int Wp diag, rest zero); block 4
    # holds root_Wp in rows 0:32 of col 0 (padded to 128 wide so the LDW
    # pull-ahead isn't blocked by a narrow stationary)
    rw2 = nc.dram_tensor("rw2", [128, (NPANEL + 1) * 128], mmdt, kind="ExternalInput")
    # all f32 per-partition bias constants in one tensor:
    # cols 0:32 leaf_b, 32:36 int_b, 36:52 leaf_bp (rows 0:8),
    # 52 int_bp (rows 0:16), 53 root_b (rows 0:32), 54 root_bp (row 0),
    # 55 = [root_b; int_bp] (rows 0:48) for the merged rh+intp tanh
    cc = nc.dram_tensor("cc", [128, 56], f32, kind="ExternalInput")
    # fp16 output staging: predictions are tanh outputs in [-1,1], so the
    # ~5e-4 fp16 quantization is well inside the error budget; halves the
    # final store drain. Host upcasts to f32.
    out = nc.dram_tensor("out", [NOUT, BC], mmdt, kind="ExternalOutput")

    mm = nc.tensor.matmul

    with tile.TileContext(nc) as tc, ExitStack() as ctx:
        consts = ctx.enter_context(tc.tile_pool(name="consts", bufs=1))
        wpool = ctx.enter_context(tc.tile_pool(name="wpool", bufs=4))
        work = ctx.enter_context(tc.tile_pool(name="work", bufs=18))
        keep = ctx.enter_context(tc.tile_pool(name="keep", bufs=1))
        psum = ctx.enter_context(tc.tile_pool(name="psum", bufs=1, space="PSUM"))

        # --- PE pre-warm: unthrottles the HAM clock gate (PE boots at
        # 1.2 GHz; ~3.4us of sustained activity -> 2.4 GHz). bf16 const
        # APs (preloaded) keep it to one MATMUL per mm (fp32 would run
        # LOW_HIGH 2-pass) and nothing gates the first one.
        warm_st = nc.const_aps.tensor(1.0, (128, 128), bf16)
        warm_mv = nc.const_aps.tensor(1.0, (128, BN), bf16)
        pwarm = psum.tile([128, BN], f32, tag="misc", bufs=1, name="pwarm")
        for _ in range(NWARM):
            mm(pwarm[:], warm_st, warm_mv, start=True, stop=True,
               skip_group_check=True)

        # --- preamble DMA issues. Hard-won constraints (traces v1-v7):
        # early supply is the binding frontier; per-DMA issue ~600ns and
        # a global ~8-deep completion-sem ring. Host-packed long-run
        # sources let each 256KB k-pair chunk move at chain rate, with
        # incremental sems so the k-outer stream starts at ~9.5us and
        # never waits. All latency-critical loads complete before the
        # 16KB-run panel loads (wp1-3, 400+GB/s, crush everything else)
        # start. cc's tiny 220B packets go after the first x chunk.
        cc_sb = consts.tile([128, 56], f32, name="cc_sb")
        # xt_sb/wp tiles are HALF-MAJOR: cols half*4096 + k*512 + c.
        # DMA packet size = min(src run, dst run), so both sides must be
        # contiguous to beat the ~140GB/s 1KB-packet ceiling.
        xt_sb = consts.tile([128, KC * BC], mmdt, name="xt_sb")
        wp0 = wpool.tile([128, KC * 1024], mmdt, tag="wpanel", name="wp0")
        for j in range(4):
            nc.scalar.dma_start(xt_sb[:, j * 1024:(j + 1) * 1024],
                                xtp[:, j * 1024:(j + 1) * 1024])
            if j == 0:
                nc.scalar.dma_start(cc_sb[:], cc[:])
            nc.sync.dma_start(wp0[:, j * 1024:(j + 1) * 1024],
                              lwh[0, :, j * 1024:(j + 1) * 1024])
        for h in range(2):
            nc.sync.dma_start(wp0[:, 4096 + h * 2048:4096 + (h + 1) * 2048],
                              lwh[0, :, 4096 + h * 2048:4096 + (h + 1) * 2048])
            nc.scalar.dma_start(xt_sb[:, 4096 + h * 2048:4096 + (h + 1) * 2048],
                                xtp[:, 4096 + h * 2048:4096 + (h + 1) * 2048])
        cw_sb = consts.tile([128, I * 2 * 128], mmdt, name="cw_sb")
        nc.sync.dma_start(cw_sb[:], cw[:])
        rw2_sb = consts.tile([128, (NPANEL + 1) * 128], mmdt, name="rw2_sb")
        wps = {0: wp0}
        for q in (1, 2, 3):
            wps[q] = wpool.tile([128, KC * 1024], mmdt, tag="wpanel", name=f"wp{q}")
            nc.sync.dma_start(wps[q][:], lwh[q])
            if q == 1:
                nc.sync.dma_start(rw2_sb[:], rw2[:])

        # scalar-engine warm: force the tanh ACT table load during the DMA
        # preamble instead of on the first real activation
        act_warm = work.tile([1, 1], f32, tag="actw", bufs=1, name="act_warm")
        nc.scalar.activation(act_warm[:], pwarm[0:1, 0:1], Tanh)

        # leaf predicts: node i at cols i*BC (+bn*BN); flushed per panel
        lp_sb = keep.tile([8, I * BC], mmdt, name="lp_sb")
        # merged root staging: rows 0:32 = root hidden (rootB moving
        # operand), rows 32:48 = int predicts — ONE tanh covers both
        # (scalar ACTIVATE has ~300ns fixed cost, so fewer+wider wins)
        rhi_sb = keep.tile([48, BC], mmdt, name="rhi_sb")
        rootp_sb = keep.tile([1, BC], mmdt, name="rootp_sb")

        inth = {}      # (panel, bn) -> [128, BN] tile: nodes 4p..4p+3 h^T
        prc1 = psum.tile([128, BN], f32, tag="prcinc", bufs=1, name="prc1")

        # deferred-op FIFO: each entry emits one PE-consuming op (a comb
        # matmul, a root contraction, a flush). Popped twice per leaf
        # group (after the 4th and 8th k-matmul) so producers' tanh
        # latency is always covered by >=0.85us of leaf streaming.
        fifo = []

        def pop_slot():
            if fifo:
                fifo.pop(0)()

        def leaf_mm(wp, gl, k, bn, pg, cols=None):
            # half-major layouts: x chunk (k, bn) at xt_sb cols
            # bn*4096 + k*512; weight (k, gl) at wp cols
            # (gl//4)*4096 + k*512 + (gl%4)*128. cols = (offset within
            # the bn half, width) for the endgame segments.
            c0 = 0 if cols is None else cols[0]
            cw_ = BN if cols is None else cols[1]
            xb = bn * 4096 + k * 512 + c0
            wb = (gl // 4) * 4096 + k * 512 + (gl % 4) * 128
            mm(
                pg[:],
                wp[:, wb:wb + 128],
                xt_sb[:, xb:xb + cw_],
                start=(k == 0),
                stop=(k == KC - 1),
            )

        def leaf_tanh(p, gl, bn, pg):
            lh = work.tile([128, BN], mmdt, tag="lh", name=f"lh{p}{bn}{gl}")
            nc.scalar.activation(
                lh[:], pg[:], Tanh, bias=cc_sb[:, GPP * p + gl:GPP * p + gl + 1]
            )
            return lh

        def comb_mm(p, il, j, lh, pcomb):
            """Fused internal-trans + leaf-predict matmul.

            pcomb rows 0:32 accumulate node (4p+il)'s hidden
            pre-activation over its two child groups; rows 32:40 pick up
            the group's 4 leaf predict dots via the block-diagonal
            columns (zeros elsewhere)."""
            i = 4 * p + il
            mm(
                pcomb[:],
                cw_sb[:, (2 * i + j) * 128:(2 * i + j + 1) * 128],
                lh[:],
                start=(j == 0),
                stop=(j == 1),
                skip_group_check=True,
            )

        def comb_post(p, il, bn, ith, pcomb):
            i = 4 * p + il
            nc.scalar.activation(
                ith[32 * il:32 * il + 32, :],
                pcomb[0:32, :],
                Tanh,
                bias=cc_sb[32 * il:32 * il + 32, 32 + p:33 + p],
            )
            nc.scalar.activation(
                lp_sb[:, i * BC + bn * BN:i * BC + bn * BN + BN],
                pcomb[32:40, :], Tanh, bias=cc_sb[0:8, 36 + i:37 + i],
            )

        def flush_lp(p, bn=None, irange=(0, 4)):
            i0, i1 = irange
            ni = i1 - i0
            if bn is None:
                nc.sync.dma_start(
                    out[32 * p + 8 * i0:32 * p + 8 * i1, :].rearrange(
                        "(i v) c -> v i c", v=8
                    ),
                    lp_sb[:, (4 * p + i0) * BC:(4 * p + i1) * BC].rearrange(
                        "v (i c) -> v i c", c=BC
                    ),
                )
            else:
                nc.sync.dma_start(
                    out[32 * p + 8 * i0:32 * p + 8 * i1,
                        bn * BN:bn * BN + BN].rearrange("(i v) c -> v i c", v=8),
                    lp_sb[:].rearrange("v (i c) -> v i c", c=BC)[
                        :, 4 * p + i0:4 * p + i1, bn * BN:bn * BN + BN
                    ],
                )

        def root_mm_inc(p, ith):
            """One panel's contribution to the bn=1 root/int-predict
            contraction, accumulated across panels in a persistent bank."""
            mm(
                prc1[:],
                rw2_sb[:, 128 * p:128 * (p + 1)],
                ith[:],
                start=(p == 0),
                stop=(p == NPANEL - 1),
                skip_group_check=True,
            )

        def root_post_mm(bn):
            prp = psum.tile([128, BN], f32, tag="pcomb", bufs=2,
                            name=f"prp{bn}")
            mm(prp[0:128, :], rw2_sb[0:32, NPANEL * 128:(NPANEL + 1) * 128],
               rhi_sb[0:32, bn * BN:bn * BN + BN], start=True, stop=True,
               skip_group_check=True)
            nc.scalar.activation(
                rootp_sb[0:1, bn * BN:bn * BN + BN], prp[0:1, :], Tanh,
                bias=cc_sb[0:1, 54:55],
            )

        # ---- comb closures --------------------------------------------
        pcombs = {}

        def make_comb(p, il, j, bn, lh, ith):
            def run():
                if j == 0:
                    pcombs[(p, il, bn)] = psum.tile(
                        [128, BN], f32, tag="pcomb", bufs=2,
                        name=f"pc{p}{bn}{il}")
                pc = pcombs[(p, il, bn)]
                comb_mm(p, il, j, lh, pc)
                if j == 1:
                    comb_post(p, il, bn, ith, pc)
            return run

        # --- panel 0: pure leaf streams first, k-outer waves for bn=0 so
        # matmuls chase the arriving x/weight chunks. No FIFO pops before
        # bn=1 (cw hasn't landed). Each group's tanh is emitted right
        # after its own k=7 matmul so the next set's bank WAR waits the
        # shortest possible scalar chain.
        ith00 = keep.tile([128, BN], mmdt, tag="inth00", name="inth00")
        for g0 in (0, 4):
            pgs = [
                psum.tile([128, BN], f32, tag="pg", bufs=4, name=f"pgko{g0}{q}")
                for q in range(4)
            ]
            lhs = {}
            for k in range(KC):
                for q in range(4):
                    leaf_mm(wp0, g0 + q, k, 0, pgs[q])
                    if k == KC - 1:
                        lhs[q] = leaf_tanh(0, g0 + q, 0, pgs[q])
            for q in range(4):
                gl = g0 + q
                fifo.append(make_comb(0, gl // 2, gl % 2, 0, lhs[q], ith00))
        inth[(0, 0)] = ith00

        ith01 = keep.tile([128, BN], mmdt, tag="inth01", name="inth01")
        for gl in range(GPP):
            pg = psum.tile([128, BN], f32, tag="pg", bufs=4, name=f"pg0b{gl}")
            for k in range(4):
                leaf_mm(wp0, gl, k, 1, pg)
            pop_slot()
            for k in range(4, KC):
                leaf_mm(wp0, gl, k, 1, pg)
            pop_slot()
            lh = leaf_tanh(0, gl, 1, pg)
            fifo.append(make_comb(0, gl // 2, gl % 2, 1, lh, ith01))
        inth[(0, 1)] = ith01

        fifo.append(lambda: root_mm_inc(0, inth[(0, 1)]))
        fifo.append(lambda: flush_lp(0))

        # --- panels 1..3. Panel 3 runs bn=1 first so its root chain
        # (incremental prc1) closes while bn=0's leaf stream still runs.
        prc0 = {}

        def prc0_partial():
            prc0["t"] = psum.tile([128, BN], f32, tag="misc", bufs=1,
                                  name="prc0")
            for q in range(NPANEL - 1):
                mm(
                    prc0["t"][:],
                    rw2_sb[:, 128 * q:128 * (q + 1)],
                    inth[(q, 0)][:],
                    start=(q == 0),
                    stop=False,
                    skip_group_check=True,
                )

        for p in range(1, NPANEL):
            wp = wps[p]
            bns = (1, 0) if p == NPANEL - 1 else (0, 1)
            for bn in bns:
                ith = keep.tile([128, BN], mmdt, tag=f"inth{p}{bn}",
                                name=f"inth{p}{bn}")
                ngl = 6 if (p == NPANEL - 1 and bn == 0) else 8
                for gl in range(ngl):
                    pg = psum.tile([128, BN], f32, tag="pg", bufs=4,
                                   name=f"pg{p}{bn}{gl}")
                    for k in range(4):
                        leaf_mm(wp, gl, k, bn, pg)
                    pop_slot()
                    for k in range(4, KC):
                        leaf_mm(wp, gl, k, bn, pg)
                    pop_slot()
                    lh = leaf_tanh(p, gl, bn, pg)
                    fifo.append(make_comb(p, gl // 2, gl % 2, bn, lh, ith))
                inth[(p, bn)] = ith

                if p < NPANEL - 1:
                    if bn == 1:
                        fifo.append(
                            lambda p=p: root_mm_inc(p, inth[(p, 1)])
                        )
                        fifo.append(lambda p=p: flush_lp(p))
                elif bn == 1:
                    # close bn1's root chain + store its halves, then open
                    # bn0's root contraction (panels 0..2 ready now). Split
                    # into one-PE-op closures so each tanh's latency hides
                    # under half a leaf group of streaming.
                    def close_a():
                        root_mm_inc(NPANEL - 1, inth[(NPANEL - 1, 1)])

                    def close_b():
                        nc.scalar.activation(
                            rhi_sb[0:48, BN:BC], prc1[0:48, :], Tanh,
                            bias=cc_sb[0:48, 55:56])

                    def close_c():
                        root_post_mm(1)
                        flush_lp(NPANEL - 1, 1)

                    def close_d():
                        nc.sync.dma_start(
                            out[L:L + I, BN:BC], rhi_sb[32:48, BN:BC]
                        )
                    fifo.append(close_a)
                    fifo.append(close_b)
                    fifo.append(close_c)
                    fifo.append(close_d)
                    fifo.append(prc0_partial)
                else:
                    # endgame: node 15 (groups 6,7) runs as a 384-col +
                    # 128-col column segment pair; each segment's root
                    # chain rides under later leaf matmuls and the
                    # merged rh+intp tanh keeps the terminal serial
                    # chain to 4 activations.
                    i3 = 4 * p + 3
                    SEGS = ((0, 384), (384, 128))
                    qcombs = {}
                    lhq = {}

                    def q_leaf(si, j):
                        cs, cn = SEGS[si]
                        gl = 6 + j
                        pg = psum.tile([128, cn], f32, tag="pg", bufs=4,
                                       name=f"pgq{si}{j}")
                        for k in range(KC):
                            leaf_mm(wp, gl, k, 0, pg, cols=(cs, cn))
                        lh = work.tile([128, cn], mmdt, tag="lh",
                                       name=f"lhq{si}{j}")
                        nc.scalar.activation(
                            lh[:], pg[:], Tanh,
                            bias=cc_sb[:, GPP * p + gl:GPP * p + gl + 1],
                        )
                        lhq[(si, j)] = lh

                    def q_comb(si):
                        cs, cn = SEGS[si]
                        qcombs[si] = psum.tile([128, cn], f32, tag="pcomb",
                                               bufs=2, name=f"pcq{si}")
                        for j in range(2):
                            mm(
                                qcombs[si][:],
                                cw_sb[:, (2 * i3 + j) * 128:(2 * i3 + j + 1) * 128],
                                lhq.pop((si, j))[:],
                                start=(j == 0),
                                stop=(j == 1),
                                skip_group_check=True,
                            )
                        nc.scalar.activation(
                            ith[96:128, cs:cs + cn], qcombs[si][0:32, :],
                            Tanh, bias=cc_sb[96:128, 32 + p:33 + p],
                        )
                        nc.scalar.activation(
                            lp_sb[:, i3 * BC + cs:i3 * BC + cs + cn],
                            qcombs[si][32:40, :], Tanh,
                            bias=cc_sb[0:8, 36 + i3:37 + i3],
                        )

                    def q_rootA(si):
                        cs, cn = SEGS[si]
                        mm(
                            prc0["t"][:, cs:cs + cn],
                            rw2_sb[:, 128 * (NPANEL - 1):128 * NPANEL],
                            ith[:, cs:cs + cn],
                            start=False,
                            stop=True,
                            skip_group_check=True,
                        )
                        nc.scalar.activation(
                            rhi_sb[0:48, cs:cs + cn],
                            prc0["t"][0:48, cs:cs + cn],
                            Tanh, bias=cc_sb[0:48, 55:56],
                        )

                    def q_rootB(si):
                        cs, cn = SEGS[si]
                        prp = psum.tile([128, cn], f32, tag="pcomb", bufs=2,
                                        name=f"prpq{si}")
                        mm(prp[0:128, :],
                           rw2_sb[0:32, NPANEL * 128:(NPANEL + 1) * 128],
                           rhi_sb[0:32, cs:cs + cn],
                           start=True, stop=True, skip_group_check=True)
                        nc.scalar.activation(
                            rootp_sb[0:1, cs:cs + cn], prp[0:1, :], Tanh,
                            bias=cc_sb[0:1, 54:55],
                        )

                    q_leaf(0, 0)
                    pop_slot()          # drains comb(il2) -> then flush
                    q_leaf(0, 1)
                    pop_slot()
                    q_leaf(1, 0)
                    q_comb(0)
                    flush_lp(NPANEL - 1, 0, irange=(0, 3))
                    q_leaf(1, 1)
                    q_rootA(0)
                    q_comb(1)
                    q_rootB(0)
                    q_rootA(1)
                    q_rootB(1)
                    # final flushes, split across both DMA queues
                    flush_lp(NPANEL - 1, 0, irange=(3, 4))
                    nc.scalar.dma_start(out[L:L + I, 0:BN],
                                        rhi_sb[32:48, 0:BN])
                    nc.sync.dma_start(out[L + I:NOUT, :], rootp_sb[:])

    nc.compile()
    return nc


def _pack_weights(inp):
    f = np.float32
    f16 = np.float16
    leaf_b = np.asarray(inp["leaf_b"], f)
    int_W = np.asarray(inp["int_W"], f)
    int_b = np.asarray(inp["int_b"], f)
    root_W = np.asarray(inp["root_W"], f)
    root_b = np.asarray(inp["root_b"], f)
    leaf_Wp = np.asarray(inp["leaf_Wp"], f)
    leaf_bp = np.asarray(inp["leaf_bp"], f)
    int_Wp = np.asarray(inp["int_Wp"], f)
    int_bp = np.asarray(inp["int_bp"], f)
    root_Wp = np.asarray(inp["root_Wp"], f)
    root_bp = np.asarray(inp["root_bp"], f)

    w = {}
    lw = np.asarray(inp["leaf_W"], f16).transpose(1, 0, 2).reshape(D, L * H)
    lwh_full = (
        lw.reshape(KC, 128, NPANEL, 1024).transpose(2, 1, 0, 3).reshape(
            NPANEL, 128, KC * 1024
        )
    )
    # wave-major repack: [P][pp][half][k][512] so panel-0's per-pair
    # DMAs are contiguous on both sides
    w["lwh"] = np.ascontiguousarray(
        lwh_full.reshape(NPANEL, 128, KC, 2, 512).transpose(0, 1, 3, 2, 4)
        .reshape(NPANEL, 128, KC * 1024)
    )

    cw = np.zeros((128, I * 2 * 128), f16)
    for i in range(I):
        for j in range(2):
            base = (2 * i + j) * 128
            # int_W chunk j of node i: rows (c*32+h) = child (4j+c) hidden h
            cw[:, base:base + 32] = int_W[i, 128 * j:128 * (j + 1), :]
            for c in range(4):
                lv = 8 * i + 4 * j + c
                cw[c * 32:(c + 1) * 32, base + 32 + 4 * j + c] = leaf_Wp[lv, :, 0]
    w["cw"] = cw

    rw2 = np.zeros((128, (NPANEL + 1) * 128), f16)
    for q in range(NPANEL):
        rw2[:, 128 * q:128 * q + 32] = root_W[128 * q:128 * (q + 1), :]
        for c in range(4):
            iv = 4 * q + c
            rw2[c * 32:(c + 1) * 32, 128 * q + 32 + 4 * q + c] = int_Wp[iv, :, 0]
    rw2[0:32, NPANEL * 128] = root_Wp[:, 0]
    w["rw2"] = rw2

    cc = np.zeros((128, 56), f)
    cc[0:32, 55] = root_b
    cc[32:48, 55] = int_bp[:, 0]
    cc[:, 0:32] = leaf_b.reshape(32, 128).T       # leaf biases, col=h, part=leaf%...
    cc[:, 32:36] = int_b.reshape(4, 128).T
    cc[0:8, 36:52] = leaf_bp.reshape(16, 8).T
    cc[0:16, 52] = int_bp[:, 0]
    cc[0:32, 53] = root_b
    cc[0, 54] = root_bp[0]
    w["cc"] = cc
    return w


def kernel(**inputs):
    from concourse.bass_utils import run_bass_kernel_spmd

    nc = _CACHE.get("nc")
    if nc is None:
        nc = _CACHE["nc"] = _build_nc()

    x = np.asarray(inputs["x"], np.float32)
    w = _pack_weights(inputs)
    in_maps = []
    for c in range(NCORES):
        m = dict(w)
        xtc = x[c * BC:(c + 1) * BC, :].T.astype(np.float16)
        v = xtc.reshape(KC, 128, 2, 512)  # [k][p][bn-half][c]
        xtp = np.empty((128, KC * 1024), np.float16)
        xtp[:, 0:4096] = v[:, :, 0, :].transpose(1, 0, 2).reshape(128, 4096)
        xtp[:, 4096:8192] = v[:, :, 1, :].transpose(1, 0, 2).reshape(128, 4096)
        m["xtp"] = xtp
        in_maps.append(m)

    res = run_bass_kernel_spmd(nc, in_maps, core_ids=list(range(NCORES)))
    _CACHE["last_res"] = res
    outs = [res.results[c]["out"] for c in range(NCORES)]
    full = np.concatenate([o[:, :, None] for o in outs], axis=1)  # [145, B, 1]
    return full.astype(np.float32)


# revision 35
# speedup vs baseline: 1.0109x; 1.0109x over previous
"""Trainium2 Bass kernel for nn_CombineNode_7395933684091 (gnn_message_passing).

Hierarchy: 128 leaf terms (each D=1024 -> H=32), 16 internal terms
(concat of 8 children hiddens, 256 -> 32), 1 root (concat of 16
internal hiddens, 512 -> 32); every term also has a 1-dim predict head.
All matmuls followed by tanh.

Strategy: data-parallel over batch across 8 cores (Bc = 1024 rows per
core), weights replicated. On-chip layout keeps hidden features on the
PARTITION axis ("h^T layout": tiles are [features, batch]), so every
level's contraction is a natural PE matmul and the child-concat is just
stacking partition tiles. x and all weights are repacked on the host so
every DMA is contiguous per partition.

Leaf level: 4 panels x 8 groups (4 leaves) x 8 k-chunk accumulated
[128,128]x[128,512] matmuls. The per-term predict heads ride along as
extra block-diagonal columns fused into the internal-level stationary
operand (cw) and the root-level stationary operand (rw2), so they cost
no extra PE streaming.

Matmul operands are float16: same PE stream rate as f32r (1 col/cycle)
but enables Fast Weight Load (fp32 disables FWL) so LDWEIGHTS hides
behind the matmul stream, and halves HBM + SBUF traffic. fp16's 10
mantissa bits keep the end-to-end max abs error ~1.6e-3 (vs 2e-2 gate).

Scheduling notes (perfetto-driven, 153.0us -> 151.2us):
- PE pre-warm uses the bf16 const AP (an fp32 const forces LOW_HIGH
  2-pass matmuls) and is sized to end when the first x/weight chunks
  land (~10.3us).
- Each dma_start costs ~600ns of issue time; early per-128KB transfers
  sustain only ~140GB/s per queue (~280 aggregate, global ~8-deep
  completion-sem ring), and the 16KB-run panel loads (400+GB/s) crush
  anything concurrent — the preamble order keeps every latency-critical
  load ahead of wp1-3 and is at the measured supply frontier. cc's tiny
  220B packets go after x-k0, not in front.
- Combine matmuls are deferred by half a leaf group (a FIFO popped
  twice per group) so the ~460ns leaf-tanh latency never stalls PE;
  this made the 43-159us stream region completely gapless.
- Scalar ACTIVATE has ~300ns fixed cost: root hidden + int predicts are
  merged into ONE 48-row tanh (rhi), and the endgame runs node 15 as
  384+128 col segments with each segment's root chain emitted under
  later leaf blocks; final flushes split across the sync+scalar queues.
"""

import numpy as np

B, D, H = 8192, 1024, 32
L, I, CPI = 128, 16, 8
NCORES = 8
BC = B // NCORES      # 1024 batch rows per core
BN = 512              # batch tile width (one PSUM bank of f32)
NBH = BC // BN        # 2 batch halves
KC = D // 128         # 8 contraction chunks for the leaf level
NPANEL = 4            # leaf panels (8 groups of 4 leaves each)
GPP = 8               # groups per panel
NOUT = L + I + 1      # 145
NWARM = 5             # pre-warm matmuls (512 cols each)

MM_DT = "float16"

_CACHE = {}


def _build_nc():
    from contextlib import ExitStack

    import concourse.mybir as mybir
    import concourse.tile as tile
    from concourse import bacc

    f32 = mybir.dt.float32
    bf16 = mybir.dt.bfloat16
    Tanh = mybir.ActivationFunctionType.Tanh
    mmdt = getattr(mybir.dt, MM_DT)

    nc = bacc.Bacc("TRN2", target_bir_lowering=False, debug=False)

    # x, host-packed partition-major so every early DMA has 2-4KB
    # contiguous runs per partition (1KB-run transfers cap at
    # ~140GB/s/queue; long-run chains reach 250-400GB/s):
    # cols 0:4096 = bn0 half, k-major ([p][k][c<512]); 4096:8192 = bn1
    xtp = nc.dram_tensor("xtp", [128, KC * 1024], mmdt, kind="ExternalInput")
    # leaf weights, wave-major per panel: [p][pp][half*4096 + k*512 +
    # gl%4*128 + c] — same layout as the wp SBUF tiles so the panel-0
    # per-pair DMAs are contiguous on BOTH sides (big packets)
    lwh = nc.dram_tensor("lwh", [NPANEL, 128, KC * 1024], mmdt,
                         kind="ExternalInput")
    # fused internal-trans + leaf-predict stationary: per (node i, chunk j)
    # a [128, 128] block: cols 0:32 int_W chunk, col 32+4j+c leaf Wp diag,
    # rest zero padding (full-width stationaries keep LDW pull-ahead alive)
    cw = nc.dram_tensor("cw", [128, I * 2 * 128], mmdt, kind="ExternalInput")
    # fused root-trans + int-predict stationary: per panel q a [128, 128]
    # block (cols 0:32 root_W chunk, 32:48 int Wp diag, rest zero); block 4
    # holds root_Wp in rows 0:32 of col 0 (padded to 128 wide so the LDW
    # pull-ahead isn't blocked by a narrow stationary)
    # block 5 duplicates block 3's rows 96:128 at base partition 0
    # (LDWEIGHTS stationary base must be 0/32/64) for the split root
    # contraction that reads node 15's hidden from the il3s scratch
    rw2 = nc.dram_tensor("rw2", [128, (NPANEL + 2) * 128], mmdt, kind="ExternalInput")
    # all f32 per-partition bias constants in one tensor:
    # cols 0:32 leaf_b, 32:36 int_b, 36:52 leaf_bp (rows 0:8),
    # 52 int_bp (rows 0:16), 53 root_b (rows 0:32), 54 root_bp (row 0),
    # 55 = [root_b; int_bp] (rows 0:48) for the merged rh+intp tanh,
    # 56 = [int_b(node15); leaf_bp(120:128)] (rows 0:40) for the merged
    # il3 hidden+predict tanh
    cc = nc.dram_tensor("cc", [128, 57], f32, kind="ExternalInput")
    # fp16 output staging: predictions are tanh outputs in [-1,1], so the
    # ~5e-4 fp16 quantization is well inside the error budget; halves the
    # final store drain. Host upcasts to f32.
    out = nc.dram_tensor("out", [NOUT, BC], mmdt, kind="ExternalOutput")

    mm = nc.tensor.matmul

    with tile.TileContext(nc) as tc, ExitStack() as ctx:
        consts = ctx.enter_context(tc.tile_pool(name="consts", bufs=1))
        wpool = ctx.enter_context(tc.tile_pool(name="wpool", bufs=4))
        work = ctx.enter_context(tc.tile_pool(name="work", bufs=18))
        keep = ctx.enter_context(tc.tile_pool(name="keep", bufs=1))
        psum = ctx.enter_context(tc.tile_pool(name="psum", bufs=1, space="PSUM"))

        # --- PE pre-warm: unthrottles the HAM clock gate (PE boots at
        # 1.2 GHz; ~3.4us of sustained activity -> 2.4 GHz). bf16 const
        # APs (preloaded) keep it to one MATMUL per mm (fp32 would run
        # LOW_HIGH 2-pass) and nothing gates the first one.
        warm_st = nc.const_aps.tensor(1.0, (128, 128), bf16)
        warm_mv = nc.const_aps.tensor(1.0, (128, BN), bf16)
        pwarm = psum.tile([128, BN], f32, tag="misc", bufs=1, name="pwarm")
        for _ in range(NWARM):
            mm(pwarm[:], warm_st, warm_mv, start=True, stop=True,
               skip_group_check=True)

        # --- preamble DMA issues. Hard-won constraints (traces v1-v7):
        # early supply is the binding frontier; per-DMA issue ~600ns and
        # a global ~8-deep completion-sem ring. Host-packed long-run
        # sources let each 256KB k-pair chunk move at chain rate, with
        # incremental sems so the k-outer stream starts at ~9.5us and
        # never waits. All latency-critical loads complete before the
        # 16KB-run panel loads (wp1-3, 400+GB/s, crush everything else)
        # start. cc's tiny 220B packets go after the first x chunk.
        cc_sb = consts.tile([128, 57], f32, name="cc_sb")
        # xt_sb/wp tiles are HALF-MAJOR: cols half*4096 + k*512 + c.
        # DMA packet size = min(src run, dst run), so both sides must be
        # contiguous to beat the ~140GB/s 1KB-packet ceiling.
        xt_sb = consts.tile([128, KC * BC], mmdt, name="xt_sb")
        wp0 = wpool.tile([128, KC * 1024], mmdt, tag="wpanel", name="wp0")
        for j in range(4):
            nc.scalar.dma_start(xt_sb[:, j * 1024:(j + 1) * 1024],
                                xtp[:, j * 1024:(j + 1) * 1024])
            if j == 0:
                nc.scalar.dma_start(cc_sb[:], cc[:])
            nc.sync.dma_start(wp0[:, j * 1024:(j + 1) * 1024],
                              lwh[0, :, j * 1024:(j + 1) * 1024])
        for h in range(2):
            nc.sync.dma_start(wp0[:, 4096 + h * 2048:4096 + (h + 1) * 2048],
                              lwh[0, :, 4096 + h * 2048:4096 + (h + 1) * 2048])
            nc.scalar.dma_start(xt_sb[:, 4096 + h * 2048:4096 + (h + 1) * 2048],
                                xtp[:, 4096 + h * 2048:4096 + (h + 1) * 2048])
        cw_sb = consts.tile([128, I * 2 * 128], mmdt, name="cw_sb")
        nc.sync.dma_start(cw_sb[:], cw[:])
        rw2_sb = consts.tile([128, (NPANEL + 2) * 128], mmdt, name="rw2_sb")
        wps = {0: wp0}
        for q in (1, 2, 3):
            wps[q] = wpool.tile([128, KC * 1024], mmdt, tag="wpanel", name=f"wp{q}")
            nc.sync.dma_start(wps[q][:], lwh[q])
            if q == 1:
                nc.sync.dma_start(rw2_sb[:], rw2[:])

        # scalar-engine warm: force the tanh ACT table load during the DMA
        # preamble instead of on the first real activation
        act_warm = work.tile([1, 1], f32, tag="actw", bufs=1, name="act_warm")
        nc.scalar.activation(act_warm[:], pwarm[0:1, 0:1], Tanh)

        # leaf predicts: node i at cols i*BC (+bn*BN); flushed per panel
        lp_sb = keep.tile([8, I * BC], mmdt, name="lp_sb")
        # merged root staging: rows 0:32 = root hidden (rootB moving
        # operand), rows 32:48 = int predicts — ONE tanh covers both
        # (scalar ACTIVATE has ~300ns fixed cost, so fewer+wider wins)
        rhi_sb = keep.tile([48, BC], mmdt, name="rhi_sb")
        rootp_sb = keep.tile([1, BC], mmdt, name="rootp_sb")

        inth = {}      # (panel, bn) -> [128, BN] tile: nodes 4p..4p+3 h^T
        prc1 = psum.tile([128, BN], f32, tag="prcinc", bufs=1, name="prc1")

        # deferred-op FIFO: each entry emits one PE-consuming op (a comb
        # matmul, a root contraction, a flush). Popped twice per leaf
        # group (after the 4th and 8th k-matmul) so producers' tanh
        # latency is always covered by >=0.85us of leaf streaming.
        fifo = []

        def pop_slot():
            if fifo:
                fifo.pop(0)()

        def leaf_mm(wp, gl, k, bn, pg, cols=None):
            # half-major layouts: x chunk (k, bn) at xt_sb cols
            # bn*4096 + k*512; weight (k, gl) at wp cols
            # (gl//4)*4096 + k*512 + (gl%4)*128. cols = (offset within
            # the bn half, width) for the endgame segments.
            c0 = 0 if cols is None else cols[0]
            cw_ = BN if cols is None else cols[1]
            xb = bn * 4096 + k * 512 + c0
            wb = (gl // 4) * 4096 + k * 512 + (gl % 4) * 128
            mm(
                pg[:],
                wp[:, wb:wb + 128],
                xt_sb[:, xb:xb + cw_],
                start=(k == 0),
                stop=(k == KC - 1),
            )

        def leaf_tanh(p, gl, bn, pg):
            lh = work.tile([128, BN], mmdt, tag="lh", name=f"lh{p}{bn}{gl}")
            nc.scalar.activation(
                lh[:], pg[:], Tanh, bias=cc_sb[:, GPP * p + gl:GPP * p + gl + 1]
            )
            return lh

        def comb_mm(p, il, j, lh, pcomb):
            """Fused internal-trans + leaf-predict matmul.

            pcomb rows 0:32 accumulate node (4p+il)'s hidden
            pre-activation over its two child groups; rows 32:40 pick up
            the group's 4 leaf predict dots via the block-diagonal
            columns (zeros elsewhere)."""
            i = 4 * p + il
            mm(
                pcomb[:],
                cw_sb[:, (2 * i + j) * 128:(2 * i + j + 1) * 128],
                lh[:],
                start=(j == 0),
                stop=(j == 1),
                skip_group_check=True,
            )

        def comb_post(p, il, bn, ith, pcomb):
            i = 4 * p + il
            nc.scalar.activation(
                ith[32 * il:32 * il + 32, :],
                pcomb[0:32, :],
                Tanh,
                bias=cc_sb[32 * il:32 * il + 32, 32 + p:33 + p],
            )
            nc.scalar.activation(
                lp_sb[:, i * BC + bn * BN:i * BC + bn * BN + BN],
                pcomb[32:40, :], Tanh, bias=cc_sb[0:8, 36 + i:37 + i],
            )

        def flush_lp(p, bn=None, irange=(0, 4)):
            i0, i1 = irange
            ni = i1 - i0
            if bn is None:
                nc.sync.dma_start(
                    out[32 * p + 8 * i0:32 * p + 8 * i1, :].rearrange(
                        "(i v) c -> v i c", v=8
                    ),
                    lp_sb[:, (4 * p + i0) * BC:(4 * p + i1) * BC].rearrange(
                        "v (i c) -> v i c", c=BC
                    ),
                )
            else:
                nc.sync.dma_start(
                    out[32 * p + 8 * i0:32 * p + 8 * i1,
                        bn * BN:bn * BN + BN].rearrange("(i v) c -> v i c", v=8),
                    lp_sb[:].rearrange("v (i c) -> v i c", c=BC)[
                        :, 4 * p + i0:4 * p + i1, bn * BN:bn * BN + BN
                    ],
                )

        def root_mm_inc(p, ith):
            """One panel's contribution to the bn=1 root/int-predict
            contraction, accumulated across panels in a persistent bank."""
            mm(
                prc1[:],
                rw2_sb[:, 128 * p:128 * (p + 1)],
                ith[:],
                start=(p == 0),
                stop=(p == NPANEL - 1),
                skip_group_check=True,
            )

        def root_post_mm(bn):
            prp = psum.tile([128, BN], f32, tag="pcomb", bufs=2,
                            name=f"prp{bn}")
            mm(prp[0:128, :], rw2_sb[0:32, NPANEL * 128:(NPANEL + 1) * 128],
               rhi_sb[0:32, bn * BN:bn * BN + BN], start=True, stop=True,
               skip_group_check=True)
            nc.scalar.activation(
                rootp_sb[0:1, bn * BN:bn * BN + BN], prp[0:1, :], Tanh,
                bias=cc_sb[0:1, 54:55],
            )

        # ---- comb closures --------------------------------------------
        pcombs = {}

        def make_comb(p, il, j, bn, lh, ith):
            def run():
                if j == 0:
                    pcombs[(p, il, bn)] = psum.tile(
                        [128, BN], f32, tag="pcomb", bufs=2,
                        name=f"pc{p}{bn}{il}")
                pc = pcombs[(p, il, bn)]
                comb_mm(p, il, j, lh, pc)
                if j == 1:
                    comb_post(p, il, bn, ith, pc)
            return run

        # --- panel 0: pure leaf streams first, k-outer waves for bn=0 so
        # matmuls chase the arriving x/weight chunks. No FIFO pops before
        # bn=1 (cw hasn't landed). Each group's tanh is emitted right
        # after its own k=7 matmul so the next set's bank WAR waits the
        # shortest possible scalar chain.
        ith00 = keep.tile([128, BN], mmdt, tag="inth00", name="inth00")
        for g0 in (0, 4):
            pgs = [
                psum.tile([128, BN], f32, tag="pg", bufs=4, name=f"pgko{g0}{q}")
                for q in range(4)
            ]
            lhs = {}
            for k in range(KC):
                for q in range(4):
                    leaf_mm(wp0, g0 + q, k, 0, pgs[q])
                    if k == KC - 1:
                        lhs[q] = leaf_tanh(0, g0 + q, 0, pgs[q])
            for q in range(4):
                gl = g0 + q
                fifo.append(make_comb(0, gl // 2, gl % 2, 0, lhs[q], ith00))
        inth[(0, 0)] = ith00

        ith01 = keep.tile([128, BN], mmdt, tag="inth01", name="inth01")
        for gl in range(GPP):
            pg = psum.tile([128, BN], f32, tag="pg", bufs=4, name=f"pg0b{gl}")
            for k in range(4):
                leaf_mm(wp0, gl, k, 1, pg)
            pop_slot()
            for k in range(4, KC):
                leaf_mm(wp0, gl, k, 1, pg)
            pop_slot()
            lh = leaf_tanh(0, gl, 1, pg)
            fifo.append(make_comb(0, gl // 2, gl % 2, 1, lh, ith01))
        inth[(0, 1)] = ith01

        fifo.append(lambda: root_mm_inc(0, inth[(0, 1)]))
        fifo.append(lambda: flush_lp(0))

        # --- panels 1..3. Panel 3 runs bn=1 first so its root chain
        # (incremental prc1) closes while bn=0's leaf stream still runs.
        prc0 = {}

        def prc0_partial():
            prc0["t"] = psum.tile([128, BN], f32, tag="misc", bufs=1,
                                  name="prc0")
            for q in range(NPANEL - 1):
                mm(
                    prc0["t"][:],
                    rw2_sb[:, 128 * q:128 * (q + 1)],
                    inth[(q, 0)][:],
                    start=(q == 0),
                    stop=False,
                    skip_group_check=True,
                )

        for p in range(1, NPANEL):
            wp = wps[p]
            bns = (1, 0) if p == NPANEL - 1 else (0, 1)
            for bn in bns:
                ith = keep.tile([128, BN], mmdt, tag=f"inth{p}{bn}",
                                name=f"inth{p}{bn}")
                ngl = 6 if (p == NPANEL - 1 and bn == 0) else 8
                for gl in range(ngl):
                    pg = psum.tile([128, BN], f32, tag="pg", bufs=4,
                                   name=f"pg{p}{bn}{gl}")
                    for k in range(4):
                        leaf_mm(wp, gl, k, bn, pg)
                    pop_slot()
                    for k in range(4, KC):
                        leaf_mm(wp, gl, k, bn, pg)
                    pop_slot()
                    lh = leaf_tanh(p, gl, bn, pg)
                    fifo.append(make_comb(p, gl // 2, gl % 2, bn, lh, ith))
                inth[(p, bn)] = ith

                if p < NPANEL - 1:
                    if bn == 1:
                        fifo.append(
                            lambda p=p: root_mm_inc(p, inth[(p, 1)])
                        )
                        fifo.append(lambda p=p: flush_lp(p))
                elif bn == 1:
                    # close bn1's root chain + store its halves, then open
                    # bn0's root contraction (panels 0..2 ready now). Split
                    # into one-PE-op closures so each tanh's latency hides
                    # under half a leaf group of streaming.
                    def close_a():
                        root_mm_inc(NPANEL - 1, inth[(NPANEL - 1, 1)])

                    def close_b():
                        nc.scalar.activation(
                            rhi_sb[0:48, BN:BC], prc1[0:48, :], Tanh,
                            bias=cc_sb[0:48, 55:56])

                    def close_c():
                        root_post_mm(1)
                        flush_lp(NPANEL - 1, 1)

                    def close_d():
                        nc.sync.dma_start(
                            out[L:L + I, BN:BC], rhi_sb[32:48, BN:BC]
                        )
                        nc.sync.dma_start(
                            out[L + I:NOUT, BN:BC], rootp_sb[0:1, BN:BC]
                        )
                    fifo.append(close_a)
                    fifo.append(close_b)
                    fifo.append(close_c)
                    fifo.append(close_d)
                    fifo.append(prc0_partial)
                else:
                    # endgame: node 15 (groups 6,7) runs as a 384-col +
                    # 128-col column segment pair. Scalar ACTIVATE has a
                    # ~300ns fixed cost, so il3's hidden+predict share
                    # ONE 40-row tanh into a scratch tile (the root
                    # contraction then reads node 15's hidden from
                    # scratch via a second accumulating matmul), and
                    # every output is flushed per segment on alternating
                    # queues so the final DMAs are tiny.
                    i3 = 4 * p + 3
                    SEGS = ((0, 384), (384, 128))
                    blk3 = slice(128 * (NPANEL - 1), 128 * NPANEL)
                    il3s = keep.tile([40, BN], mmdt, name="il3s")
                    qcombs = {}
                    lhq = {}

                    def q_leaf(si, j):
                        cs, cn = SEGS[si]
                        gl = 6 + j
                        pg = psum.tile([128, cn], f32, tag="pg", bufs=4,
                                       name=f"pgq{si}{j}")
                        for k in range(KC):
                            leaf_mm(wp, gl, k, 0, pg, cols=(cs, cn))
                        lh = work.tile([128, cn], mmdt, tag="lh",
                                       name=f"lhq{si}{j}")
                        nc.scalar.activation(
                            lh[:], pg[:], Tanh,
                            bias=cc_sb[:, GPP * p + gl:GPP * p + gl + 1],
                        )
                        lhq[(si, j)] = lh

                    def q_comb(si):
                        cs, cn = SEGS[si]
                        qcombs[si] = psum.tile([128, cn], f32, tag="pcomb",
                                               bufs=2, name=f"pcq{si}")
                        for j in range(2):
                            mm(
                                qcombs[si][:],
                                cw_sb[:, (2 * i3 + j) * 128:(2 * i3 + j + 1) * 128],
                                lhq.pop((si, j))[:],
                                start=(j == 0),
                                stop=(j == 1),
                                skip_group_check=True,
                            )
                        nc.scalar.activation(
                            il3s[0:40, cs:cs + cn], qcombs[si][0:40, :],
                            Tanh, bias=cc_sb[0:40, 56:57],
                        )
                        nc.sync.dma_start(
                            out[32 * p + 24:32 * p + 32, cs:cs + cn],
                            il3s[32:40, cs:cs + cn],
                        )

                    def q_rootA(si):
                        cs, cn = SEGS[si]
                        mm(
                            prc0["t"][:, cs:cs + cn],
                            rw2_sb[0:96, blk3],
                            ith[0:96, cs:cs + cn],
                            start=False,
                            stop=False,
                            skip_group_check=True,
                        )
                        mm(
                            prc0["t"][:, cs:cs + cn],
                            rw2_sb[0:32, 128 * (NPANEL + 1):128 * (NPANEL + 2)],
                            il3s[0:32, cs:cs + cn],
                            start=False,
                            stop=True,
                            skip_group_check=True,
                        )
                        nc.scalar.activation(
                            rhi_sb[0:48, cs:cs + cn],
                            prc0["t"][0:48, cs:cs + cn],
                            Tanh, bias=cc_sb[0:48, 55:56],
                        )

                    def q_rootB(si):
                        cs, cn = SEGS[si]
                        prp = psum.tile([128, cn], f32, tag="pcomb", bufs=2,
                                        name=f"prpq{si}")
                        mm(prp[0:128, :],
                           rw2_sb[0:32, NPANEL * 128:(NPANEL + 1) * 128],
                           rhi_sb[0:32, cs:cs + cn],
                           start=True, stop=True, skip_group_check=True)
                        nc.scalar.activation(
                            rootp_sb[0:1, cs:cs + cn], prp[0:1, :], Tanh,
                            bias=cc_sb[0:1, 54:55],
                        )
                        nc.scalar.dma_start(out[L:L + I, cs:cs + cn],
                                            rhi_sb[32:48, cs:cs + cn])
                        nc.sync.dma_start(out[L + I:NOUT, cs:cs + cn],
                                          rootp_sb[0:1, cs:cs + cn])

                    q_leaf(0, 0)
                    pop_slot()          # drains comb(il2) -> then flush
                    q_leaf(0, 1)
                    pop_slot()
                    q_leaf(1, 0)
                    q_comb(0)
                    flush_lp(NPANEL - 1, 0, irange=(0, 3))
                    q_leaf(1, 1)
                    q_rootA(0)
                    q_comb(1)
                    q_rootB(0)
                    q_rootA(1)
                    q_rootB(1)

    nc.compile()
    return nc


def _pack_weights(inp):
    f = np.float32
    f16 = np.float16
    leaf_b = np.asarray(inp["leaf_b"], f)
    int_W = np.asarray(inp["int_W"], f)
    int_b = np.asarray(inp["int_b"], f)
    root_W = np.asarray(inp["root_W"], f)
    root_b = np.asarray(inp["root_b"], f)
    leaf_Wp = np.asarray(inp["leaf_Wp"], f)
    leaf_bp = np.asarray(inp["leaf_bp"], f)
    int_Wp = np.asarray(inp["int_Wp"], f)
    int_bp = np.asarray(inp["int_bp"], f)
    root_Wp = np.asarray(inp["root_Wp"], f)
    root_bp = np.asarray(inp["root_bp"], f)

    w = {}
    lw = np.asarray(inp["leaf_W"], f16).transpose(1, 0, 2).reshape(D, L * H)
    lwh_full = (
        lw.reshape(KC, 128, NPANEL, 1024).transpose(2, 1, 0, 3).reshape(
            NPANEL, 128, KC * 1024
        )
    )
    # wave-major repack: [P][pp][half][k][512] so panel-0's per-pair
    # DMAs are contiguous on both sides
    w["lwh"] = np.ascontiguousarray(
        lwh_full.reshape(NPANEL, 128, KC, 2, 512).transpose(0, 1, 3, 2, 4)
        .reshape(NPANEL, 128, KC * 1024)
    )

    cw = np.zeros((128, I * 2 * 128), f16)
    for i in range(I):
        for j in range(2):
            base = (2 * i + j) * 128
            # int_W chunk j of node i: rows (c*32+h) = child (4j+c) hidden h
            cw[:, base:base + 32] = int_W[i, 128 * j:128 * (j + 1), :]
            for c in range(4):
                lv = 8 * i + 4 * j + c
                cw[c * 32:(c + 1) * 32, base + 32 + 4 * j + c] = leaf_Wp[lv, :, 0]
    w["cw"] = cw

    rw2 = np.zeros((128, (NPANEL + 2) * 128), f16)
    for q in range(NPANEL):
        rw2[:, 128 * q:128 * q + 32] = root_W[128 * q:128 * (q + 1), :]
        for c in range(4):
            iv = 4 * q + c
            rw2[c * 32:(c + 1) * 32, 128 * q + 32 + 4 * q + c] = int_Wp[iv, :, 0]
    rw2[0:32, NPANEL * 128] = root_Wp[:, 0]
    rw2[0:32, (NPANEL + 1) * 128:(NPANEL + 2) * 128] = \
        rw2[96:128, (NPANEL - 1) * 128:NPANEL * 128]
    w["rw2"] = rw2

    cc = np.zeros((128, 57), f)
    cc[0:32, 55] = root_b
    cc[32:48, 55] = int_bp[:, 0]
    cc[0:32, 56] = int_b[15]
    cc[32:40, 56] = leaf_bp.reshape(16, 8).T[:, 15]
    cc[:, 0:32] = leaf_b.reshape(32, 128).T       # leaf biases, col=h, part=leaf%...
    cc[:, 32:36] = int_b.reshape(4, 128).T
    cc[0:8, 36:52] = leaf_bp.reshape(16, 8).T
    cc[0:16, 52] = int_bp[:, 0]
    cc[0:32, 53] = root_b
    cc[0, 54] = root_bp[0]
    w["cc"] = cc
    return w


def kernel(**inputs):
    from concourse.bass_utils import run_bass_kernel_spmd

    nc = _CACHE.get("nc")
    if nc is None:
        nc = _CACHE["nc"] = _build_nc()

    x = np.asarray(inputs["x"], np.float32)
    w = _pack_weights(inputs)
    in_maps = []
    for c in range(NCORES):
        m = dict(w)
        xtc = x[c * BC:(c + 1) * BC, :].T.astype(np.float16)
        v = xtc.reshape(KC, 128, 2, 512)  # [k][p][bn-half][c]
        xtp = np.empty((128, KC * 1024), np.float16)
        xtp[:, 0:4096] = v[:, :, 0, :].transpose(1, 0, 2).reshape(128, 4096)
        xtp[:, 4096:8192] = v[:, :, 1, :].transpose(1, 0, 2).reshape(128, 4096)
        m["xtp"] = xtp
        in_maps.append(m)

    res = run_bass_kernel_spmd(nc, in_maps, core_ids=list(range(NCORES)))
    _CACHE["last_res"] = res
    outs = [res.results[c]["out"] for c in range(NCORES)]
    full = np.concatenate([o[:, :, None] for o in outs], axis=1)  # [145, B, 1]
    return full.astype(np.float32)
